# revision 9
# baseline (speedup 1.0000x reference)
"""Trainium2 Bass kernel for nn_DiffKS (differentiable Karplus-Strong).

Structure:
  1. Frame-rate params (250 frames) are upsampled to sample rate with natural
     cubic splines on the host (float64); per-sample 3-tap IIR coefficients
     (g1,g2,g3) and integer delays z in [89, 317] are derived.
  2. The strictly sequential recursion
         y[t] = x[t] + g1*y[t-z-1] + g2*y[t-z-2] + g3*y[t-z-3]
     runs on the tensor engine in chunks of W=128 samples.  Tap lags are
     >= 90, so within a chunk the dependency matrix L is nilpotent (L^2=0)
     and is folded on the host into the cross-chunk weight blocks via
     T = I + L^T.  Each chunk is then 1-2 dense [128,128] blocks against
     previous chunk columns of the signal, evaluated as bf16 hi/lo matmuls
     (Whi@yhi + Whi@ylo + Wlo@yhi) accumulating in fp32 PSUM.
  3. LDWEIGHTS bandwidth dominates (FWL gives 2 bf16/cycle for the 128-col
     stationaries).  Per block the hi weights serve two matmuls; a hybrid
     emission order (hi-pair first for old-source blocks, ylo-matmul last
     for the freshest block) keeps every (hi,hi) pair adjacent on the PE
     stream without stalling on the previous chunk's eviction, and a
     post-pass drops the redundant loads (3 -> 2 per block).
  4. The excitation (first 2048 samples) is folded through T on the host and
     added during PSUM eviction of the first 16 chunks.
"""

import re

import ml_dtypes
import numpy as np

import concourse.bass as bass
import concourse.mybir as mybir
import concourse.tile as tile
from concourse import bacc
from concourse.bass_utils import run_bass_kernel_spmd

F32 = mybir.dt.float32
BF16 = mybir.dt.bfloat16
BF16NP = ml_dtypes.bfloat16
N_CORES = 8
BG = 32  # bf16 weight slots per DMA group (2 slots per logical block)


# ----------------------------------------------------------------- host math
def _host_preprocess(delay_frames, raw_coeff, excitation, n_samples):
    dt = np.float64
    F = delay_frames.shape[0]
    sig = 1.0 / (1.0 + np.exp(-raw_coeff.astype(dt)))
    coeff = sig / sig.sum(-1, keepdims=True)
    t_in = np.linspace(0.0, 1.0, F).astype(dt)
    t_out = np.linspace(0.0, 1.0, n_samples).astype(dt)
    x = np.concatenate([delay_frames.astype(dt)[:, None], coeff], axis=1)
    h = t_in[1:] - t_in[:-1]
    hinv = 1.0 / h
    dx3 = 3.0 * (x[1:] - x[:-1])
    rhs_part = dx3 * (hinv * hinv)[:, None]
    diag = np.zeros(F, dt)
    diag[:-1] += hinv
    diag[1:] += hinv
    diag *= 2.0
    rhs = np.zeros_like(x)
    rhs[:-1] += rhs_part
    rhs[1:] += rhs_part
    M = np.diag(diag) + np.diag(hinv, 1) + np.diag(hinv, -1)
    k = np.linalg.solve(M, rhs)
    hc = hinv[:, None]
    a = x[:-1]
    b = k[:-1]
    two_c = (2.0 * dx3 * hc - 4.0 * k[:-1] - 2.0 * k[1:]) * hc
    three_d = (-2.0 * dx3 * hc + 3.0 * (k[:-1] + k[1:])) * hc * hc
    idx = np.clip(np.searchsorted(t_in, t_out, side="left") - 1, 0, F - 2)
    f = (t_out - t_in[idx])[:, None]
    inner = b[idx] + (0.5 * two_c[idx] + three_d[idx] * (f / 3.0)) * f
    vals = a[idx] + inner * f
    delay = vals[:, 0]
    b1 = vals[:, 1]
    b2 = vals[:, 2]
    zf = np.floor(delay)
    z = zf.astype(np.int64)
    alfa = delay - zf
    g1 = b1 * (1.0 - alfa)
    g2 = b1 * alfa + b2 * (1.0 - alfa)
    g3 = b2 * alfa
    xfull = np.zeros(n_samples, np.float64)
    nx = min(excitation.shape[0], n_samples)
    xfull[:nx] = excitation[:nx]
    return z, g1, g2, g3, xfull


def _build_blocks_folded(z, g1, g2, g3, xfull, n, W):
    """Cross-chunk blocks B~_c[src_row, out_col] with the within-chunk
    (nilpotent) dependency matrix folded in, plus folded excitation
    columns."""
    minlag = int(z.min()) + 1
    assert 2 * minlag > W and n % W == 0, (minlag, W, n)
    i1 = np.arange(n) - z - 1
    n_chunks = n // W
    blocks = []
    chunk_cols = []
    xt = np.zeros((W, n_chunks), np.float64)
    for m in range(n_chunks):
        s0 = m * W
        per_c = {}
        L = None
        for j, g in ((0, g1), (1, g2), (2, g3)):
            for t in range(s0, s0 + W):
                i = i1[t] - j
                if i < 0:
                    continue
                c = m - i // W
                if c == 0:
                    if L is None:
                        L = np.zeros((W, W), np.float64)
                    L[i % W, t - s0] += g[t]
                    continue
                blk = per_c.get(c)
                if blk is None:
                    blk = per_c[c] = np.zeros((W, W), np.float64)
                blk[i % W, t - s0] += g[t]
        xm = xfull[s0 : s0 + W].copy()
        if L is not None:
            for c in per_c:
                per_c[c] = per_c[c] + per_c[c] @ L
            xm = xm + L.T @ xm
        xt[:, m] = xm
        if not per_c:
            per_c[1] = np.zeros((W, W), np.float64)
        cs = sorted(per_c.keys(), reverse=True)  # oldest column first
        chunk_cols.append(cs)
        blocks.extend(per_c[c] for c in cs)
    return blocks, chunk_cols, xt


def _pack_weights(blocks, W):
    nslots = 2 * len(blocks)
    ngroups = (nslots + BG - 1) // BG
    wts = np.zeros((ngroups, W, BG * W), BF16NP)
    for i, b in enumerate(blocks):
        b32 = b.astype(np.float32)
        hi = b32.astype(BF16NP)
        lo = (b32 - hi.astype(np.float32)).astype(BF16NP)
        g, off = divmod(2 * i, BG)
        wts[g, :, off * W : (off + 1) * W] = hi
        wts[g, :, (off + 1) * W : (off + 2) * W] = lo
    return wts, ngroups


# ------------------------------------------------------- LDW dedup post-pass
def _ldw_key(inst):
    m = re.search(r"in=\[(.*?)\]\s*tile", inst.concise())
    return m.group(1) if m else None


def _remove_redundant_ldweights(nc):
    """Drop LDWEIGHTS whose weights AP equals the immediately preceding
    LDWEIGHTS on the PE stream (the intervening matmuls leave the PE array
    untouched).  Waits/updates carried by a dropped load move onto the next
    matmul: waits still guard the data it reads, updates still fire after
    the array is done with the weights."""
    removed = 0
    for f in nc.m.functions:
        for blk in f.blocks:
            prev_key = None
            pending_waits = []
            pending_updates = []
            new_list = []
            changed = False
            for inst in blk.instructions:
                tn = type(inst).__name__
                if tn == "InstLdweights":
                    key = _ldw_key(inst)
                    si = inst.sync_info
                    if key is not None and key == prev_key:
                        if si is not None:
                            pending_waits.extend(si.on_wait)
                            pending_updates.extend(si.on_update)
                        removed += 1
                        changed = True
                        continue
                    prev_key = key
                elif tn == "InstMatmult":
                    if pending_waits or pending_updates:
                        msi = inst.sync_info
                        if msi is None:
                            inst.sync_info = mybir.SyncInfo(
                                on_wait=list(pending_waits),
                                on_update=list(pending_updates),
                            )
                        else:
                            msi.on_wait = list(msi.on_wait) + pending_waits
                            msi.on_update = (
                                list(msi.on_update) + pending_updates
                            )
                        pending_waits = []
                        pending_updates = []
                elif inst.engine == mybir.EngineType.PE:
                    prev_key = None
                new_list.append(inst)
            assert not pending_waits and not pending_updates
            if changed:
                _replace_instructions(blk, new_list)
    return removed


def _replace_instructions(blk, new_list):
    try:
        blk.instructions = new_list
        return
    except (AttributeError, TypeError):
        pass
    old = list(blk.instructions)
    keep = set(id(i) for i in new_list)
    for inst in old:
        if id(inst) not in keep:
            blk.instructions.remove(inst)


# ------------------------------------------------------------- device kernel
def _build_nc(n_chunks, chunk_cols, ngroups, W, lead, n_x_chunks):
    nc = bacc.Bacc(
        "TRN2", target_bir_lowering=False, debug=False, num_devices=N_CORES
    )
    wts = nc.dram_tensor("wts", [ngroups, W, BG * W], BF16, kind="ExternalInput")
    xin = nc.dram_tensor("xin", [W, max(n_x_chunks, 1)], F32, kind="ExternalInput")
    yout = nc.dram_tensor("yout", [W, n_chunks], F32, kind="ExternalOutput")
    ncols = lead + n_chunks
    with tile.TileContext(nc) as tc:
        with (
            tc.tile_pool(name="ybuf", bufs=1) as ypool,
            tc.tile_pool(name="wpool", bufs=10) as wpool,
            tc.tile_pool(name="psum", bufs=8, space="PSUM") as ppool,
        ):
            # interleaved hi/lo signal columns: [W, col, 2]
            yhl = ypool.tile([W, ncols, 2], BF16, tag="yhl")
            nc.vector.memset(yhl[:, :, :], 0.0)
            xs = ypool.tile([W, max(n_x_chunks, 1)], F32, tag="xs")
            nc.sync.dma_start(out=xs[:, :], in_=xin[:, :])
            tmp = ypool.tile([W, 1], F32, tag="tmp")
            bi = 0
            wt = None
            for m in range(n_chunks):
                psum = ppool.tile([W, 1], F32, tag="acc")
                cs = chunk_cols[m]
                nblk = len(cs)
                for k, c in enumerate(cs):
                    g, off = divmod(bi, BG)
                    if off == 0:
                        wt = wpool.tile([W, BG * W], BF16)
                        # three partition-slices on the three DMA rings so
                        # group fetches don't serialize behind one ring
                        r1 = W // 3
                        r2 = 2 * (W // 3)
                        nc.sync.dma_start(out=wt[0:r1, :], in_=wts[g, 0:r1])
                        nc.scalar.dma_start(out=wt[r1:r2, :], in_=wts[g, r1:r2])
                        nc.gpsimd.dma_start(out=wt[r2:W, :], in_=wts[g, r2:W])
                    whi = wt[0:W, off * W : (off + 1) * W]
                    wlo = wt[0:W, (off + 1) * W : (off + 2) * W]
                    col = lead + m - c
                    # three bf16 products accumulate into one psum column.
                    # Hybrid order: blocks against old columns (c >= 2, all
                    # operands long ready) lead with the (hi,hi) pair so the
                    # scheduler keeps it adjacent and the post-pass drops the
                    # repeated load; the freshest block (c == 1) trails its
                    # ylo-dependent matmul so it needn't wait for the
                    # previous chunk's second eviction op.
                    if c >= 2:
                        nc.tensor.matmul(
                            psum[:, 0:1], lhsT=whi, rhs=yhl[0:W, col, 1:2],
                            start=(k == 0), stop=False, skip_group_check=True,
                        )
                        nc.tensor.matmul(
                            psum[:, 0:1], lhsT=whi, rhs=yhl[0:W, col, 0:1],
                            start=False, stop=False, skip_group_check=True,
                        )
                        nc.tensor.matmul(
                            psum[:, 0:1], lhsT=wlo, rhs=yhl[0:W, col, 0:1],
                            start=False, stop=(k == nblk - 1),
                            skip_group_check=True,
                        )
                    else:
                        nc.tensor.matmul(
                            psum[:, 0:1], lhsT=wlo, rhs=yhl[0:W, col, 0:1],
                            start=(k == 0), stop=False, skip_group_check=True,
                        )
                        nc.tensor.matmul(
                            psum[:, 0:1], lhsT=whi, rhs=yhl[0:W, col, 0:1],
                            start=False, stop=False, skip_group_check=True,
                        )
                        nc.tensor.matmul(
                            psum[:, 0:1], lhsT=whi, rhs=yhl[0:W, col, 1:2],
                            start=False, stop=(k == nblk - 1),
                            skip_group_check=True,
                        )
                    bi += 2
                mcol = lead + m
                # eviction on DVE: yhi = bf16(psum (+x)); ylo = psum - yhi
                if m < n_x_chunks:
                    nc.vector.tensor_add(tmp[:, :], psum[:, 0:1], xs[:, m : m + 1])
                    nc.vector.tensor_copy(yhl[0:W, mcol, 0:1], tmp[:, :])
                    nc.vector.tensor_sub(
                        yhl[0:W, mcol, 1:2], tmp[:, :], yhl[0:W, mcol, 0:1]
                    )
                else:
                    nc.vector.tensor_copy(yhl[0:W, mcol, 0:1], psum[:, 0:1])
                    nc.vector.tensor_sub(
                        yhl[0:W, mcol, 1:2], psum[:, 0:1], yhl[0:W, mcol, 0:1]
                    )
            ysum = ypool.tile([W, n_chunks], F32, tag="ysum")
            nc.vector.tensor_add(
                ysum[:, :], yhl[0:W, lead:ncols, 0], yhl[0:W, lead:ncols, 1]
            )
            nc.sync.dma_start(out=yout[:, :], in_=ysum[:, :])
    _remove_redundant_ldweights(nc)
    nc.compile()
    return nc


_LAST_RESULT = {}


def kernel(delay_len_frames, raw_coeff_frames, excitation, n_samples):
    n = int(n_samples)
    z, g1, g2, g3, xfull = _host_preprocess(
        np.asarray(delay_len_frames), np.asarray(raw_coeff_frames),
        np.asarray(excitation), n,
    )
    # chunk width: as large as possible (<=128) subject to the fold-validity
    # condition 2*(min tap lag) > W and W | n
    W = 128
    while W > 1 and not (2 * (int(z.min()) + 1) > W and n % W == 0):
        W //= 2
    lead = -(-(int(z.max()) + 3) // W)
    blocks, chunk_cols, xt = _build_blocks_folded(z, g1, g2, g3, xfull, n, W)
    n_chunks = n // W
    nx = int(np.asarray(excitation).shape[0])
    n_x_chunks = min(-(-nx // W), n_chunks)
    wts, ngroups = _pack_weights(blocks, W)
    xin = xt[:, : max(n_x_chunks, 1)].astype(np.float32)

    nc = _build_nc(n_chunks, chunk_cols, ngroups, W, lead, n_x_chunks)
    import os

    in_map = {"wts": wts, "xin": xin}
    res = run_bass_kernel_spmd(
        nc,
        [in_map] * N_CORES,
        core_ids=list(range(N_CORES)),
        trace=os.environ.get("DIFFKS_TRACE", "") not in ("", "0"),
    )
    _LAST_RESULT["res"] = res
    ycols = res.results[0]["yout"]  # [W, n_chunks]
    y = ycols.T.reshape(-1)[:n].astype(np.float32)
    return y
